# revision 7
# baseline (speedup 1.0000x reference)
"""DKVMN (DeepIRT) forward pass on 8 Trainium2 NeuronCores.

Strategy (v3)
-------------
Pure data parallel over the batch (2048 -> 256 per core, 2 partition-tiles
of 128). Token-dependent quantities are folded into gather tables on the
host (weight-only preprocessing):

  Wsoft[q]  = softmax(q_embed @ key_memory^T)   (attention weights w)
  Hq[q]     = q_embed @ pred_w1[V:] + b1        (query part of the MLP)
  Esig[qa]  = sigmoid(qa_embed @ erase_w + be)  (erase gate e)
  Atanh[qa] = tanh(qa_embed @ add_w + ba)       (add vector a)

DVE tensor_tensor runs 2x-mode ONLY on flat contiguous APs (5.3us per
[128,10000] fp16 pass); broadcast APs drop it to 1x (~19.6us).  So ACT
(which runs 1x always, 8.3us/pass, broadcast-tolerant) materializes the
broadcast operands and every DVE op is flat:

  ACT    : WR  = broadcast(w) along V   [P,M,V]       (8.6us)
           E10 = e tiled 10x            [P,2000]      (2.0us)
  GPSIMD : X   = WR * a_bc (middle-stride-0 AP, ~22us, overlapped)
  DVE    : T1 = Mv*WR (flat 2x)
           erase: 5 chunks {ysc=T1c*E10; Mvc-=ysc}  ([P,2000] flat 2x)
           add-tree on T1 -> read; MLP smalls
           Mv += X LAST (so GPSIMD's X latency is hidden)
  PE     : tiny prediction matmuls/transposes
"""

import os
import sys

for _p in ("/root/.axon_site/_ro/trn_rl_repo", "/opt/trn_rl_repo"):
    if os.path.isdir(_p) and _p not in sys.path:
        sys.path.append(_p)

import numpy as np

import concourse.bacc as bacc
import concourse.bass as bass
import concourse.tile as tile
from concourse import mybir
from concourse.bass_utils import run_bass_kernel_spmd
from concourse.masks import make_identity

# Problem shapes (hardcoded per harness contract)
B, S, M, V, KD, FC = 2048, 200, 50, 200, 50, 50
NQ, NQA = 5001, 10001
NCORES = 8
BL = B // NCORES      # 256 batch rows per core
P = 128               # SBUF partitions
NT = BL // P          # 2 batch tiles per core
KSTEPS = 2            # time steps per gather block
NBLK = S // KSTEPS
MV = M * V            # 10000
REP = 10              # e replication factor (built on ACT)
CHW = REP * V         # 2000: erase chunk width (10 slots)
NCH = M // REP        # 5 chunks per erase
EAW = 512             # ea-table row width (fp16 elems); 1024B, %256 ok
WHW = 128             # wh-table row width; 256B
IDX_PER_BLK = BL * KSTEPS        # 512 gathered rows per block per table
IDXCOLS = BL * S // 16           # wrapped idx array columns

_prog_cache = {}


def _build_program(steps=S):
    dt = mybir.dt
    nc = bacc.Bacc("TRN2", debug=False)

    ea_t = nc.dram_tensor("ea_table", [NQA, EAW], dt.float16, kind="ExternalInput")
    wh_t = nc.dram_tensor("wh_table", [NQ, WHW], dt.float16, kind="ExternalInput")
    w1r_d = nc.dram_tensor("w1r", [2, 100, FC], dt.float16, kind="ExternalInput")
    w2_d = nc.dram_tensor("w2rep", [P, FC], dt.float16, kind="ExternalInput")
    b2_d = nc.dram_tensor("b2rep", [P, 1], dt.float32, kind="ExternalInput")
    mv_d = nc.dram_tensor("mv_init", [1, MV], dt.float16, kind="ExternalInput")
    qi_d = nc.dram_tensor("qidx", [P, IDXCOLS], dt.int16, kind="ExternalInput")
    qa_d = nc.dram_tensor("qaidx", [P, IDXCOLS], dt.int16, kind="ExternalInput")
    preds_d = nc.dram_tensor("preds_out", [BL, S], dt.float32, kind="ExternalOutput")

    nblk = steps // KSTEPS

    from contextlib import ExitStack

    with tile.TileContext(nc) as tc, ExitStack() as ctx:
        consts = ctx.enter_context(tc.tile_pool(name="consts", bufs=1))
        state = ctx.enter_context(tc.tile_pool(name="state", bufs=1))
        gath = ctx.enter_context(tc.tile_pool(name="gath", bufs=2))
        ysc_p = ctx.enter_context(tc.tile_pool(name="ysc", bufs=1))
        small = ctx.enter_context(tc.tile_pool(name="small", bufs=2))
        psum = ctx.enter_context(tc.tile_pool(name="psum", bufs=2, space="PSUM"))

        # ---- constants ----
        w1r_sb = consts.tile([100, 2, FC], dt.float16)
        for c in range(2):
            nc.sync.dma_start(out=w1r_sb[:, c, :], in_=w1r_d[c])
        w2_sb = consts.tile([P, FC], dt.float16)
        nc.sync.dma_start(out=w2_sb[:], in_=w2_d[:])
        b2_sb = consts.tile([P, 1], dt.float32)
        nc.sync.dma_start(out=b2_sb[:], in_=b2_d[:])
        ident = consts.tile([P, P], dt.float16)
        make_identity(nc, ident)

        # ---- persistent state (flat [P, MV] tensors) ----
        Mvs, T1s, Xs, WRs, E10s = [], [], [], [], []
        for tl in range(NT):
            Mv = state.tile([P, MV], dt.float16, tag=f"mv{tl}", name=f"mv{tl}")
            nc.sync.dma_start(out=Mv[:], in_=mv_d[:].to_broadcast((P, MV)))
            Mvs.append(Mv)
            T1s.append(state.tile([P, MV], dt.float16, tag=f"t1{tl}", name=f"t1{tl}"))
            Xs.append(state.tile([P, MV], dt.float16, tag=f"x{tl}", name=f"x{tl}"))
            WRs.append(state.tile([P, MV], dt.float16, tag=f"wr{tl}", name=f"wr{tl}"))
            E10s.append(state.tile([P, CHW], dt.float16, tag=f"e10{tl}", name=f"e10{tl}"))
        preds_buf = state.tile([P, NT, S], dt.float32, tag="preds")

        mult = mybir.AluOpType.mult
        addop = mybir.AluOpType.add

        # ---- scan ----
        for g in range(nblk):
            qi = gath.tile([P, IDX_PER_BLK // 16], dt.int16, tag="qi")
            qa = gath.tile([P, IDX_PER_BLK // 16], dt.int16, tag="qa")
            c0 = g * (IDX_PER_BLK // 16)
            nc.sync.dma_start(out=qi[:], in_=qi_d[:, c0:c0 + IDX_PER_BLK // 16])
            nc.sync.dma_start(out=qa[:], in_=qa_d[:, c0:c0 + IDX_PER_BLK // 16])
            ea_blk = gath.tile([P, NT * KSTEPS, EAW], dt.float16, tag="ea")
            wh_blk = gath.tile([P, NT * KSTEPS, WHW], dt.float16, tag="wh")
            nc.gpsimd.dma_gather(ea_blk[:], ea_t[:], qa[:], IDX_PER_BLK, IDX_PER_BLK, EAW)
            nc.gpsimd.dma_gather(wh_blk[:], wh_t[:], qi[:], IDX_PER_BLK, IDX_PER_BLK, WHW)

            for k in range(KSTEPS):
                t = g * KSTEPS + k
                for tl in range(NT):
                    c = k * NT + tl
                    w_sl = wh_blk[:, c, 0:M]
                    hq_sl = wh_blk[:, c, 64:64 + FC]
                    e_sl = ea_blk[:, c, 0:V]
                    a_sl = ea_blk[:, c, 256:256 + V]
                    Mv, T1, X, WR, E10 = Mvs[tl], T1s[tl], Xs[tl], WRs[tl], E10s[tl]

                    # ACT: replicate w along v -> WR [P, M, V]; tile e 10x -> E10
                    nc.scalar.copy(
                        WR[:].rearrange("p (m v) -> p m v", m=M),
                        w_sl[:, :, None].to_broadcast((P, M, V)))
                    nc.scalar.copy(
                        E10[:].rearrange("p (j v) -> p j v", j=REP),
                        e_sl[:, None, :].to_broadcast((P, REP, V)))
                    # GPSIMD: X = WR * a_bc (middle-stride-0 src1, ~22us)
                    nc.gpsimd.tensor_mul(
                        X[:].rearrange("p (m v) -> p m v", m=M),
                        WR[:].rearrange("p (m v) -> p m v", m=M),
                        a_sl[:, None, :].to_broadcast((P, M, V)))
                    # DVE: T1 = Mv * WR  (flat 2x)
                    nc.vector.tensor_mul(T1[:], Mv[:], WR[:])
                    # DVE: chunked erase, all flat slices (2x mode)
                    for ch in range(NCH):
                        lo, hi = ch * CHW, (ch + 1) * CHW
                        ysc = ysc_p.tile([P, CHW], dt.float16, tag="ysc")
                        nc.vector.tensor_mul(ysc[:], T1[:, lo:hi], E10[:])
                        nc.vector.tensor_sub(Mv[:, lo:hi], Mv[:, lo:hi], ysc[:])

                    # DVE: add-tree over m on T1 (in place) -> read [P, V]
                    read = small.tile([P, V], dt.float16, tag="read")
                    nc.vector.tensor_add(T1[:, 0:5000], T1[:, 0:5000], T1[:, 5000:10000])
                    nc.vector.tensor_add(T1[:, 0:2400], T1[:, 0:2400], T1[:, 2400:4800])
                    nc.vector.tensor_add(T1[:, 0:1200], T1[:, 0:1200], T1[:, 1200:2400])
                    nc.vector.tensor_add(T1[:, 0:600], T1[:, 0:600], T1[:, 600:1200])
                    nc.vector.tensor_add(T1[:, 0:200], T1[:, 0:200], T1[:, 200:400])
                    nc.vector.tensor_add(T1[:, 0:200], T1[:, 0:200], T1[:, 400:600])
                    nc.vector.tensor_add(read[:], T1[:, 0:200], T1[:, 4800:5000])

                    # PE: h = read @ W1r   (transpose read, 2 K-chunks of 100)
                    readT = small.tile([100, 2, P], dt.float16, tag="readT")
                    for cc in range(2):
                        pT = psum.tile([100, P], dt.float16, tag="pT")
                        nc.tensor.transpose(pT[:], read[:, cc * 100:(cc + 1) * 100], ident[:])
                        nc.scalar.copy(readT[:, cc, :], pT[:])
                    h_ps = psum.tile([P, FC], dt.float32, tag="hps")
                    nc.tensor.matmul(h_ps[:], lhsT=readT[:, 0, :], rhs=w1r_sb[:, 0, :],
                                     start=True, stop=False)
                    nc.tensor.matmul(h_ps[:], lhsT=readT[:, 1, :], rhs=w1r_sb[:, 1, :],
                                     start=False, stop=True)
                    hpre = small.tile([P, FC], dt.float16, tag="hpre")
                    nc.vector.tensor_add(hpre[:], h_ps[:], hq_sl)
                    hact = small.tile([P, FC], dt.float16, tag="hact")
                    nc.scalar.activation(hact[:], hpre[:], mybir.ActivationFunctionType.Tanh)
                    hw2 = small.tile([P, FC], dt.float16, tag="hw2")
                    pacc = small.tile([P, 1], dt.float32, tag="pacc")
                    nc.vector.tensor_mul(hw2[:], hact[:], w2_sb[:])
                    nc.vector.tensor_reduce(pacc[:], hw2[:], mybir.AxisListType.X, addop)
                    nc.scalar.activation(
                        preds_buf[:, tl, t:t + 1], pacc[:],
                        mybir.ActivationFunctionType.Sigmoid, bias=b2_sb[:],
                    )

                    # DVE: Mv += X LAST (lets GPSIMD's X-outer-product overlap)
                    nc.vector.tensor_add(Mv[:], Mv[:], X[:])

        # ---- write out ----
        pv = preds_d[:].rearrange("(n p) s -> n p s", p=P)
        for tl in range(NT):
            nc.sync.dma_start(out=pv[tl][:, 0:steps], in_=preds_buf[:, tl, 0:steps])

    nc.finalize()
    return nc


def _wrap_idx(seq):
    """seq [N] -> [128, N//16] int16 wrapped (idx i at [i%16, i//16], 8x replicated)."""
    n = seq.shape[0]
    arr16 = seq.reshape(n // 16, 16).T.astype(np.int16)
    return np.tile(arr16, (8, 1))


def _host_tables(inputs):
    f32 = np.float32
    qe = inputs["q_embed_w"].astype(f32)
    qae = inputs["qa_embed_w"].astype(f32)
    km = inputs["key_memory"].astype(f32)

    logits = qe @ km.T
    ex = np.exp(logits - logits.max(-1, keepdims=True))
    wsoft = ex / ex.sum(-1, keepdims=True)
    hq = qe @ inputs["pred_w1"][V:, :].astype(f32) + inputs["pred_b1"].astype(f32)
    esig = 1.0 / (1.0 + np.exp(-(qae @ inputs["erase_w"].astype(f32) + inputs["erase_b"].astype(f32))))
    atanh = np.tanh(qae @ inputs["add_w"].astype(f32) + inputs["add_b"].astype(f32))

    ea = np.zeros((NQA, EAW), np.float16)
    ea[:, 0:V] = esig.astype(np.float16)
    ea[:, 256:256 + V] = atanh.astype(np.float16)
    wh = np.zeros((NQ, WHW), np.float16)
    wh[:, 0:M] = wsoft.astype(np.float16)
    wh[:, 64:64 + FC] = hq.astype(np.float16)

    w1r = inputs["pred_w1"][:V, :].astype(np.float16).reshape(2, 100, FC)
    w2rep = np.tile(inputs["pred_w2"][:, 0].astype(np.float16)[None, :], (P, 1))
    b2rep = np.full((P, 1), inputs["pred_b2"][0], np.float32)
    mv_init = inputs["init_value_memory"].astype(np.float16).reshape(1, -1)
    return dict(ea_table=ea, wh_table=wh, w1r=w1r, w2rep=w2rep, b2rep=b2rep,
                mv_init=mv_init)


def kernel(**inputs):
    inputs = {k: np.asarray(v) for k, v in inputs.items()}
    steps = int(os.environ.get("KERNEL_STEPS", S))

    if steps not in _prog_cache:
        _prog_cache[steps] = _build_program(steps)
    nc = _prog_cache[steps]

    shared = _host_tables(inputs)
    q = inputs["q_data"].astype(np.int64)
    qa = inputs["qa_data"].astype(np.int64)

    in_maps = []
    for core in range(NCORES):
        qs = q[core * BL:(core + 1) * BL]       # [256, S]
        qas = qa[core * BL:(core + 1) * BL]
        # gather order: block g, step k, tile tl, partition p
        def order(x):
            xt = x.T.reshape(S, NT, P)
            return xt.reshape(NBLK, KSTEPS, NT, P).reshape(-1)
        m = dict(shared)
        m["qidx"] = _wrap_idx(order(qs))
        m["qaidx"] = _wrap_idx(order(qas))
        in_maps.append(m)

    trace = bool(int(os.environ.get("KERNEL_TRACE", "0")))
    res = run_bass_kernel_spmd(nc, in_maps, core_ids=list(range(NCORES)), trace=trace)
    global LAST_RESULTS
    LAST_RESULTS = res
    preds = np.concatenate(
        [res.results[i]["preds_out"] for i in range(NCORES)], axis=0
    ).astype(np.float32)
    z = np.zeros_like(preds)
    return (preds, z, z, z)


# revision 8
# speedup vs baseline: 1.0063x; 1.0063x over previous
"""DKVMN (DeepIRT) forward pass on 8 Trainium2 NeuronCores.

Strategy (v3)
-------------
Pure data parallel over the batch (2048 -> 256 per core, 2 partition-tiles
of 128). Token-dependent quantities are folded into gather tables on the
host (weight-only preprocessing):

  Wsoft[q]  = softmax(q_embed @ key_memory^T)   (attention weights w)
  Hq[q]     = q_embed @ pred_w1[V:] + b1        (query part of the MLP)
  Esig[qa]  = sigmoid(qa_embed @ erase_w + be)  (erase gate e)
  Atanh[qa] = tanh(qa_embed @ add_w + ba)       (add vector a)

DVE tensor_tensor runs 2x-mode ONLY on flat contiguous APs (5.3us per
[128,10000] fp16 pass); broadcast APs drop it to 1x (~19.6us).  So ACT
(which runs 1x always, 8.3us/pass, broadcast-tolerant) materializes the
broadcast operands and every DVE op is flat:

  ACT    : WR  = broadcast(w) along V   [P,M,V]       (8.6us)
           E10 = e tiled 10x            [P,2000]      (2.0us)
  GPSIMD : X   = WR * a_bc (middle-stride-0 AP, ~22us, overlapped)
  DVE    : T1 = Mv*WR (flat 2x)
           erase: 5 chunks {ysc=T1c*E10; Mvc-=ysc}  ([P,2000] flat 2x)
           add-tree on T1 -> read; MLP smalls
           Mv += X LAST (so GPSIMD's X latency is hidden)
  PE     : tiny prediction matmuls/transposes
"""

import os
import sys

for _p in ("/root/.axon_site/_ro/trn_rl_repo", "/opt/trn_rl_repo"):
    if os.path.isdir(_p) and _p not in sys.path:
        sys.path.append(_p)

import numpy as np

import concourse.bacc as bacc
import concourse.bass as bass
import concourse.tile as tile
from concourse import mybir
from concourse.bass_utils import run_bass_kernel_spmd
from concourse.masks import make_identity

# Problem shapes (hardcoded per harness contract)
B, S, M, V, KD, FC = 2048, 200, 50, 200, 50, 50
NQ, NQA = 5001, 10001
NCORES = 8
BL = B // NCORES      # 256 batch rows per core
P = 128               # SBUF partitions
NT = BL // P          # 2 batch tiles per core
KSTEPS = 2            # time steps per gather block
NBLK = S // KSTEPS
MV = M * V            # 10000
REP = 10              # e replication factor (built on ACT)
CHW = REP * V         # 2000: erase chunk width (10 slots)
NCH = M // REP        # 5 chunks per erase
EAW = 512             # ea-table row width (fp16 elems); 1024B, %256 ok
WHW = 128             # wh-table row width; 256B
IDX_PER_BLK = BL * KSTEPS        # 512 gathered rows per block per table
IDXCOLS = BL * S // 16           # wrapped idx array columns

_prog_cache = {}


def _build_program(steps=S):
    dt = mybir.dt
    nc = bacc.Bacc("TRN2", debug=False)

    ea_t = nc.dram_tensor("ea_table", [NQA, EAW], dt.float16, kind="ExternalInput")
    wh_t = nc.dram_tensor("wh_table", [NQ, WHW], dt.float16, kind="ExternalInput")
    w1r_d = nc.dram_tensor("w1r", [2, 100, FC], dt.float16, kind="ExternalInput")
    w2_d = nc.dram_tensor("w2rep", [P, FC], dt.float16, kind="ExternalInput")
    b2_d = nc.dram_tensor("b2rep", [P, 1], dt.float32, kind="ExternalInput")
    mv_d = nc.dram_tensor("mv_init", [1, MV], dt.float16, kind="ExternalInput")
    qi_d = nc.dram_tensor("qidx", [P, IDXCOLS], dt.int16, kind="ExternalInput")
    qa_d = nc.dram_tensor("qaidx", [P, IDXCOLS], dt.int16, kind="ExternalInput")
    preds_d = nc.dram_tensor("preds_out", [BL, S], dt.float32, kind="ExternalOutput")

    nblk = steps // KSTEPS

    from contextlib import ExitStack

    with tile.TileContext(nc) as tc, ExitStack() as ctx:
        consts = ctx.enter_context(tc.tile_pool(name="consts", bufs=1))
        state = ctx.enter_context(tc.tile_pool(name="state", bufs=1))
        gath = ctx.enter_context(tc.tile_pool(name="gath", bufs=2))
        ysc_p = ctx.enter_context(tc.tile_pool(name="ysc", bufs=1))
        small = ctx.enter_context(tc.tile_pool(name="small", bufs=2))
        psum = ctx.enter_context(tc.tile_pool(name="psum", bufs=2, space="PSUM"))

        # ---- constants ----
        w1r_sb = consts.tile([100, 2, FC], dt.float16)
        for c in range(2):
            nc.sync.dma_start(out=w1r_sb[:, c, :], in_=w1r_d[c])
        w2_sb = consts.tile([P, FC], dt.float16)
        nc.sync.dma_start(out=w2_sb[:], in_=w2_d[:])
        b2_sb = consts.tile([P, 1], dt.float32)
        nc.sync.dma_start(out=b2_sb[:], in_=b2_d[:])
        ident = consts.tile([P, P], dt.float16)
        make_identity(nc, ident)

        # ---- persistent state (flat [P, MV] tensors) ----
        Mvs, T1s, Xs, WRs, E10s = [], [], [], [], []
        for tl in range(NT):
            Mv = state.tile([P, MV], dt.float16, tag=f"mv{tl}", name=f"mv{tl}")
            nc.sync.dma_start(out=Mv[:], in_=mv_d[:].to_broadcast((P, MV)))
            Mvs.append(Mv)
            T1s.append(state.tile([P, MV], dt.float16, tag=f"t1{tl}", name=f"t1{tl}"))
            Xs.append(state.tile([P, MV], dt.float16, tag=f"x{tl}", name=f"x{tl}"))
            WRs.append(state.tile([P, MV], dt.float16, tag=f"wr{tl}", name=f"wr{tl}"))
            E10s.append(state.tile([P, CHW], dt.float16, tag=f"e10{tl}", name=f"e10{tl}"))
        preds_buf = state.tile([P, NT, S], dt.float32, tag="preds")

        mult = mybir.AluOpType.mult
        addop = mybir.AluOpType.add

        # ---- scan ----
        for g in range(nblk):
            qi = gath.tile([P, IDX_PER_BLK // 16], dt.int16, tag="qi")
            qa = gath.tile([P, IDX_PER_BLK // 16], dt.int16, tag="qa")
            c0 = g * (IDX_PER_BLK // 16)
            nc.sync.dma_start(out=qi[:], in_=qi_d[:, c0:c0 + IDX_PER_BLK // 16])
            nc.sync.dma_start(out=qa[:], in_=qa_d[:, c0:c0 + IDX_PER_BLK // 16])
            ea_blk = gath.tile([P, NT * KSTEPS, EAW], dt.float16, tag="ea")
            wh_blk = gath.tile([P, NT * KSTEPS, WHW], dt.float16, tag="wh")
            nc.gpsimd.dma_gather(ea_blk[:], ea_t[:], qa[:], IDX_PER_BLK, IDX_PER_BLK, EAW)
            nc.gpsimd.dma_gather(wh_blk[:], wh_t[:], qi[:], IDX_PER_BLK, IDX_PER_BLK, WHW)

            for k in range(KSTEPS):
                t = g * KSTEPS + k
                for tl in range(NT):
                    c = k * NT + tl
                    w_sl = wh_blk[:, c, 0:M]
                    hq_sl = wh_blk[:, c, 64:64 + FC]
                    e_sl = ea_blk[:, c, 0:V]
                    a_sl = ea_blk[:, c, 256:256 + V]
                    Mv, T1, X, WR, E10 = Mvs[tl], T1s[tl], Xs[tl], WRs[tl], E10s[tl]

                    # ACT: replicate w along v -> WR [P, M, V]; tile e 10x -> E10
                    nc.scalar.copy(
                        WR[:].rearrange("p (m v) -> p m v", m=M),
                        w_sl[:, :, None].to_broadcast((P, M, V)))
                    nc.scalar.copy(
                        E10[:].rearrange("p (j v) -> p j v", j=REP),
                        e_sl[:, None, :].to_broadcast((P, REP, V)))
                    # GPSIMD: X = WR * a_bc in 5 chunks (shorter ops so the
                    # DVE queue is never blocked behind a 20us GPSIMD op)
                    for xc in range(5):
                        xlo = xc * (MV // 5)
                        nc.gpsimd.tensor_mul(
                            X[:, xlo:xlo + MV // 5].rearrange("p (m v) -> p m v", m=M // 5),
                            WR[:, xlo:xlo + MV // 5].rearrange("p (m v) -> p m v", m=M // 5),
                            a_sl[:, None, :].to_broadcast((P, M // 5, V)))
                    # DVE: T1 = Mv * WR  (flat 2x)
                    nc.vector.tensor_mul(T1[:], Mv[:], WR[:])
                    # DVE: chunked erase, all flat slices (2x mode)
                    for ch in range(NCH):
                        lo, hi = ch * CHW, (ch + 1) * CHW
                        ysc = ysc_p.tile([P, CHW], dt.float16, tag="ysc")
                        nc.vector.tensor_mul(ysc[:], T1[:, lo:hi], E10[:])
                        nc.vector.tensor_sub(Mv[:, lo:hi], Mv[:, lo:hi], ysc[:])

                    # DVE: add-tree over m on T1 (in place) -> read [P, V]
                    read = small.tile([P, V], dt.float16, tag="read")
                    nc.vector.tensor_add(T1[:, 0:5000], T1[:, 0:5000], T1[:, 5000:10000])
                    nc.vector.tensor_add(T1[:, 0:2400], T1[:, 0:2400], T1[:, 2400:4800])
                    nc.vector.tensor_add(T1[:, 0:1200], T1[:, 0:1200], T1[:, 1200:2400])
                    nc.vector.tensor_add(T1[:, 0:600], T1[:, 0:600], T1[:, 600:1200])
                    nc.vector.tensor_add(T1[:, 0:200], T1[:, 0:200], T1[:, 200:400])
                    nc.vector.tensor_add(T1[:, 0:200], T1[:, 0:200], T1[:, 400:600])
                    nc.vector.tensor_add(read[:], T1[:, 0:200], T1[:, 4800:5000])

                    # PE: h = read @ W1r   (transpose read, 2 K-chunks of 100)
                    readT = small.tile([100, 2, P], dt.float16, tag="readT")
                    for cc in range(2):
                        pT = psum.tile([100, P], dt.float16, tag="pT")
                        nc.tensor.transpose(pT[:], read[:, cc * 100:(cc + 1) * 100], ident[:])
                        nc.scalar.copy(readT[:, cc, :], pT[:])
                    h_ps = psum.tile([P, FC], dt.float32, tag="hps")
                    nc.tensor.matmul(h_ps[:], lhsT=readT[:, 0, :], rhs=w1r_sb[:, 0, :],
                                     start=True, stop=False)
                    nc.tensor.matmul(h_ps[:], lhsT=readT[:, 1, :], rhs=w1r_sb[:, 1, :],
                                     start=False, stop=True)
                    hpre = small.tile([P, FC], dt.float16, tag="hpre")
                    nc.vector.tensor_add(hpre[:], h_ps[:], hq_sl)
                    hact = small.tile([P, FC], dt.float16, tag="hact")
                    nc.scalar.activation(hact[:], hpre[:], mybir.ActivationFunctionType.Tanh)
                    hw2 = small.tile([P, FC], dt.float16, tag="hw2")
                    pacc = small.tile([P, 1], dt.float32, tag="pacc")
                    nc.vector.tensor_mul(hw2[:], hact[:], w2_sb[:])
                    nc.vector.tensor_reduce(pacc[:], hw2[:], mybir.AxisListType.X, addop)
                    nc.scalar.activation(
                        preds_buf[:, tl, t:t + 1], pacc[:],
                        mybir.ActivationFunctionType.Sigmoid, bias=b2_sb[:],
                    )

                    # DVE: Mv += X LAST (lets GPSIMD's X-outer-product overlap)
                    nc.vector.tensor_add(Mv[:], Mv[:], X[:])

        # ---- write out ----
        pv = preds_d[:].rearrange("(n p) s -> n p s", p=P)
        for tl in range(NT):
            nc.sync.dma_start(out=pv[tl][:, 0:steps], in_=preds_buf[:, tl, 0:steps])

    nc.finalize()
    return nc


def _wrap_idx(seq):
    """seq [N] -> [128, N//16] int16 wrapped (idx i at [i%16, i//16], 8x replicated)."""
    n = seq.shape[0]
    arr16 = seq.reshape(n // 16, 16).T.astype(np.int16)
    return np.tile(arr16, (8, 1))


def _host_tables(inputs):
    f32 = np.float32
    qe = inputs["q_embed_w"].astype(f32)
    qae = inputs["qa_embed_w"].astype(f32)
    km = inputs["key_memory"].astype(f32)

    logits = qe @ km.T
    ex = np.exp(logits - logits.max(-1, keepdims=True))
    wsoft = ex / ex.sum(-1, keepdims=True)
    hq = qe @ inputs["pred_w1"][V:, :].astype(f32) + inputs["pred_b1"].astype(f32)
    esig = 1.0 / (1.0 + np.exp(-(qae @ inputs["erase_w"].astype(f32) + inputs["erase_b"].astype(f32))))
    atanh = np.tanh(qae @ inputs["add_w"].astype(f32) + inputs["add_b"].astype(f32))

    ea = np.zeros((NQA, EAW), np.float16)
    ea[:, 0:V] = esig.astype(np.float16)
    ea[:, 256:256 + V] = atanh.astype(np.float16)
    wh = np.zeros((NQ, WHW), np.float16)
    wh[:, 0:M] = wsoft.astype(np.float16)
    wh[:, 64:64 + FC] = hq.astype(np.float16)

    w1r = inputs["pred_w1"][:V, :].astype(np.float16).reshape(2, 100, FC)
    w2rep = np.tile(inputs["pred_w2"][:, 0].astype(np.float16)[None, :], (P, 1))
    b2rep = np.full((P, 1), inputs["pred_b2"][0], np.float32)
    mv_init = inputs["init_value_memory"].astype(np.float16).reshape(1, -1)
    return dict(ea_table=ea, wh_table=wh, w1r=w1r, w2rep=w2rep, b2rep=b2rep,
                mv_init=mv_init)


def kernel(**inputs):
    inputs = {k: np.asarray(v) for k, v in inputs.items()}
    steps = int(os.environ.get("KERNEL_STEPS", S))

    if steps not in _prog_cache:
        _prog_cache[steps] = _build_program(steps)
    nc = _prog_cache[steps]

    shared = _host_tables(inputs)
    q = inputs["q_data"].astype(np.int64)
    qa = inputs["qa_data"].astype(np.int64)

    in_maps = []
    for core in range(NCORES):
        qs = q[core * BL:(core + 1) * BL]       # [256, S]
        qas = qa[core * BL:(core + 1) * BL]
        # gather order: block g, step k, tile tl, partition p
        def order(x):
            xt = x.T.reshape(S, NT, P)
            return xt.reshape(NBLK, KSTEPS, NT, P).reshape(-1)
        m = dict(shared)
        m["qidx"] = _wrap_idx(order(qs))
        m["qaidx"] = _wrap_idx(order(qas))
        in_maps.append(m)

    trace = bool(int(os.environ.get("KERNEL_TRACE", "0")))
    res = run_bass_kernel_spmd(nc, in_maps, core_ids=list(range(NCORES)), trace=trace)
    global LAST_RESULTS
    LAST_RESULTS = res
    preds = np.concatenate(
        [res.results[i]["preds_out"] for i in range(NCORES)], axis=0
    ).astype(np.float32)
    z = np.zeros_like(preds)
    return (preds, z, z, z)


# revision 11
# speedup vs baseline: 1.5060x; 1.4965x over previous
"""DKVMN (DeepIRT) forward pass on 8 Trainium2 NeuronCores.

Strategy (v3)
-------------
Pure data parallel over the batch (2048 -> 256 per core, 2 partition-tiles
of 128). Token-dependent quantities are folded into gather tables on the
host (weight-only preprocessing):

  Wsoft[q]  = softmax(q_embed @ key_memory^T)   (attention weights w)
  Hq[q]     = q_embed @ pred_w1[V:] + b1        (query part of the MLP)
  Esig[qa]  = sigmoid(qa_embed @ erase_w + be)  (erase gate e)
  Atanh[qa] = tanh(qa_embed @ add_w + ba)       (add vector a)

DVE tensor_tensor runs 2x-mode ONLY on flat contiguous APs (5.3us per
[128,10000] fp16 pass); broadcast APs drop it to 1x (~19.6us).  So ACT
(which runs 1x always, 8.3us/pass, broadcast-tolerant) materializes the
broadcast operands and every DVE op is flat:

  ACT    : WR  = broadcast(w) along V   [P,M,V]       (8.6us)
           E10 = e tiled 10x            [P,2000]      (2.0us)
  GPSIMD : X   = WR * a_bc (middle-stride-0 AP, ~22us, overlapped)
  DVE    : T1 = Mv*WR (flat 2x)
           erase: 5 chunks {ysc=T1c*E10; Mvc-=ysc}  ([P,2000] flat 2x)
           add-tree on T1 -> read; MLP smalls
           Mv += X LAST (so GPSIMD's X latency is hidden)
  PE     : tiny prediction matmuls/transposes
"""

import os
import sys

for _p in ("/root/.axon_site/_ro/trn_rl_repo", "/opt/trn_rl_repo"):
    if os.path.isdir(_p) and _p not in sys.path:
        sys.path.append(_p)

import numpy as np

import concourse.bacc as bacc
import concourse.bass as bass
import concourse.tile as tile
from concourse import mybir
from concourse.bass_utils import run_bass_kernel_spmd
from concourse.masks import make_identity

# Problem shapes (hardcoded per harness contract)
B, S, M, V, KD, FC = 2048, 200, 50, 200, 50, 50
NQ, NQA = 5001, 10001
NCORES = 8
BL = B // NCORES      # 256 batch rows per core
P = 128               # SBUF partitions
NT = BL // P          # 2 batch tiles per core
KSTEPS = 2            # time steps per gather block
NBLK = S // KSTEPS
MV = M * V            # 10000
REP = 10              # e replication factor (built on ACT)
CHW = REP * V         # 2000: erase chunk width (10 slots)
NCH = M // REP        # 5 chunks per erase
EAW = 512             # ea-table row width (fp16 elems); 1024B, %256 ok
WHW = 128             # wh-table row width; 256B
IDX_PER_BLK = BL * KSTEPS        # 512 gathered rows per block per table
IDXCOLS = BL * S // 16           # wrapped idx array columns
KACT = 41                        # X slots built on ACT (rest on DVE)

_prog_cache = {}


def _build_program(steps=S):
    dt = mybir.dt
    nc = bacc.Bacc("TRN2", debug=False)

    ea_t = nc.dram_tensor("ea_table", [NQA, EAW], dt.float16, kind="ExternalInput")
    wh_t = nc.dram_tensor("wh_table", [NQ, WHW], dt.float16, kind="ExternalInput")
    w1r_d = nc.dram_tensor("w1r", [2, 100, FC], dt.float16, kind="ExternalInput")
    w2_d = nc.dram_tensor("w2rep", [P, FC], dt.float16, kind="ExternalInput")
    b2_d = nc.dram_tensor("b2rep", [P, 1], dt.float32, kind="ExternalInput")
    mv_d = nc.dram_tensor("mv_init", [1, MV], dt.float16, kind="ExternalInput")
    qi_d = nc.dram_tensor("qidx", [P, IDXCOLS], dt.int16, kind="ExternalInput")
    qa_d = nc.dram_tensor("qaidx", [P, IDXCOLS], dt.int16, kind="ExternalInput")
    preds_d = nc.dram_tensor("preds_out", [BL, S], dt.float32, kind="ExternalOutput")

    nblk = steps // KSTEPS

    from contextlib import ExitStack

    with tile.TileContext(nc) as tc, ExitStack() as ctx:
        consts = ctx.enter_context(tc.tile_pool(name="consts", bufs=1))
        state = ctx.enter_context(tc.tile_pool(name="state", bufs=1))
        gath = ctx.enter_context(tc.tile_pool(name="gath", bufs=2))
        ysc_p = ctx.enter_context(tc.tile_pool(name="ysc", bufs=1))
        small = ctx.enter_context(tc.tile_pool(name="small", bufs=2))
        psum = ctx.enter_context(tc.tile_pool(name="psum", bufs=2, space="PSUM"))

        # ---- constants ----
        w1r_sb = consts.tile([100, 2, FC], dt.float16)
        for c in range(2):
            nc.sync.dma_start(out=w1r_sb[:, c, :], in_=w1r_d[c])
        w2_sb = consts.tile([P, FC], dt.float16)
        nc.sync.dma_start(out=w2_sb[:], in_=w2_d[:])
        b2_sb = consts.tile([P, 1], dt.float32)
        nc.sync.dma_start(out=b2_sb[:], in_=b2_d[:])
        ident = consts.tile([P, P], dt.float16)
        make_identity(nc, ident)

        # ---- persistent state (flat [P, MV] tensors) ----
        Mvs, T1s, Xs, WRs, E10s = [], [], [], [], []
        for tl in range(NT):
            Mv = state.tile([P, MV], dt.float16, tag=f"mv{tl}", name=f"mv{tl}")
            nc.sync.dma_start(out=Mv[:], in_=mv_d[:].to_broadcast((P, MV)))
            Mvs.append(Mv)
            T1s.append(state.tile([P, MV], dt.float16, tag=f"t1{tl}", name=f"t1{tl}"))
            Xs.append(state.tile([P, MV], dt.float16, tag=f"x{tl}", name=f"x{tl}"))
            WRs.append(state.tile([P, MV], dt.float16, tag=f"wr{tl}", name=f"wr{tl}"))
            E10s.append(state.tile([P, CHW], dt.float16, tag=f"e10{tl}", name=f"e10{tl}"))
        preds_buf = state.tile([P, NT, S], dt.float32, tag="preds")

        mult = mybir.AluOpType.mult
        addop = mybir.AluOpType.add

        # ---- scan ----
        for g in range(nblk):
            qi = gath.tile([P, IDX_PER_BLK // 16], dt.int16, tag="qi")
            qa = gath.tile([P, IDX_PER_BLK // 16], dt.int16, tag="qa")
            c0 = g * (IDX_PER_BLK // 16)
            nc.sync.dma_start(out=qi[:], in_=qi_d[:, c0:c0 + IDX_PER_BLK // 16])
            nc.sync.dma_start(out=qa[:], in_=qa_d[:, c0:c0 + IDX_PER_BLK // 16])
            ea_blk = gath.tile([P, NT * KSTEPS, EAW], dt.float16, tag="ea")
            wh_blk = gath.tile([P, NT * KSTEPS, WHW], dt.float16, tag="wh")
            nc.gpsimd.dma_gather(ea_blk[:], ea_t[:], qa[:], IDX_PER_BLK, IDX_PER_BLK, EAW)
            nc.gpsimd.dma_gather(wh_blk[:], wh_t[:], qi[:], IDX_PER_BLK, IDX_PER_BLK, WHW)

            for k in range(KSTEPS):
                t = g * KSTEPS + k
                for tl in range(NT):
                    c = k * NT + tl
                    w_sl = wh_blk[:, c, 0:M]
                    hq_sl = wh_blk[:, c, 64:64 + FC]
                    e_sl = ea_blk[:, c, 0:V]
                    a_sl = ea_blk[:, c, 256:256 + V]
                    Mv, T1, X, WR, E10 = Mvs[tl], T1s[tl], Xs[tl], WRs[tl], E10s[tl]

                    # ACT: replicate w along v -> WR [P, M, V]; tile e 10x -> E10
                    # (GPSIMD does NO elementwise work: concurrent GPSIMD TT
                    # steals SBUF bandwidth and slows DVE ops 2.5-4x)
                    nc.scalar.copy(
                        WR[:].rearrange("p (m v) -> p m v", m=M),
                        w_sl[:, :, None].to_broadcast((P, M, V)))
                    nc.scalar.copy(
                        E10[:].rearrange("p (j v) -> p j v", j=REP),
                        e_sl[:, None, :].to_broadcast((P, REP, V)))
                    # X = w (x) a built per-slot: ACT does slots [0, KACT) via
                    # Copy with per-partition scale; DVE does the tail via
                    # 4x-mode tensor_scalar_mul. Scalar APs must be fp32.
                    wf = small.tile([P, M], dt.float32, tag="wf")
                    nc.scalar.copy(wf[:], w_sl)
                    for m in range(KACT):
                        nc.scalar.activation(
                            X[:, m * V:(m + 1) * V], a_sl,
                            mybir.ActivationFunctionType.Copy,
                            scale=wf[:, m:m + 1])
                    for m in range(KACT, M):
                        nc.vector.tensor_scalar_mul(
                            X[:, m * V:(m + 1) * V], a_sl, wf[:, m:m + 1])
                    # DVE: T1 = Mv * WR  (flat 2x)
                    nc.vector.tensor_mul(T1[:], Mv[:], WR[:])
                    # DVE: chunked erase, all flat slices (2x mode)
                    for ch in range(NCH):
                        lo, hi = ch * CHW, (ch + 1) * CHW
                        ysc = ysc_p.tile([P, CHW], dt.float16, tag="ysc")
                        nc.vector.tensor_mul(ysc[:], T1[:, lo:hi], E10[:])
                        nc.vector.tensor_sub(Mv[:, lo:hi], Mv[:, lo:hi], ysc[:])

                    # DVE: add-tree over m on T1 (in place) -> read [P, V]
                    read = small.tile([P, V], dt.float16, tag="read")
                    nc.vector.tensor_add(T1[:, 0:5000], T1[:, 0:5000], T1[:, 5000:10000])
                    nc.vector.tensor_add(T1[:, 0:2400], T1[:, 0:2400], T1[:, 2400:4800])
                    nc.vector.tensor_add(T1[:, 0:1200], T1[:, 0:1200], T1[:, 1200:2400])
                    nc.vector.tensor_add(T1[:, 0:600], T1[:, 0:600], T1[:, 600:1200])
                    nc.vector.tensor_add(T1[:, 0:200], T1[:, 0:200], T1[:, 200:400])
                    nc.vector.tensor_add(T1[:, 0:200], T1[:, 0:200], T1[:, 400:600])
                    nc.vector.tensor_add(read[:], T1[:, 0:200], T1[:, 4800:5000])

                    # PE: h = read @ W1r   (transpose read, 2 K-chunks of 100)
                    readT = small.tile([100, 2, P], dt.float16, tag="readT")
                    for cc in range(2):
                        pT = psum.tile([100, P], dt.float16, tag="pT")
                        nc.tensor.transpose(pT[:], read[:, cc * 100:(cc + 1) * 100], ident[:])
                        nc.scalar.copy(readT[:, cc, :], pT[:])
                    h_ps = psum.tile([P, FC], dt.float32, tag="hps")
                    nc.tensor.matmul(h_ps[:], lhsT=readT[:, 0, :], rhs=w1r_sb[:, 0, :],
                                     start=True, stop=False)
                    nc.tensor.matmul(h_ps[:], lhsT=readT[:, 1, :], rhs=w1r_sb[:, 1, :],
                                     start=False, stop=True)
                    hpre = small.tile([P, FC], dt.float16, tag="hpre")
                    nc.vector.tensor_add(hpre[:], h_ps[:], hq_sl)
                    hact = small.tile([P, FC], dt.float16, tag="hact")
                    nc.scalar.activation(hact[:], hpre[:], mybir.ActivationFunctionType.Tanh)
                    hw2 = small.tile([P, FC], dt.float16, tag="hw2")
                    pacc = small.tile([P, 1], dt.float32, tag="pacc")
                    nc.vector.tensor_mul(hw2[:], hact[:], w2_sb[:])
                    nc.vector.tensor_reduce(pacc[:], hw2[:], mybir.AxisListType.X, addop)
                    nc.scalar.activation(
                        preds_buf[:, tl, t:t + 1], pacc[:],
                        mybir.ActivationFunctionType.Sigmoid, bias=b2_sb[:],
                    )

                    # DVE: Mv += X LAST (lets GPSIMD's X-outer-product overlap)
                    nc.vector.tensor_add(Mv[:], Mv[:], X[:])

        # ---- write out ----
        pv = preds_d[:].rearrange("(n p) s -> n p s", p=P)
        for tl in range(NT):
            nc.sync.dma_start(out=pv[tl][:, 0:steps], in_=preds_buf[:, tl, 0:steps])

    nc.finalize()
    return nc


def _wrap_idx(seq):
    """seq [N] -> [128, N//16] int16 wrapped (idx i at [i%16, i//16], 8x replicated)."""
    n = seq.shape[0]
    arr16 = seq.reshape(n // 16, 16).T.astype(np.int16)
    return np.tile(arr16, (8, 1))


def _host_tables(inputs):
    f32 = np.float32
    qe = inputs["q_embed_w"].astype(f32)
    qae = inputs["qa_embed_w"].astype(f32)
    km = inputs["key_memory"].astype(f32)

    logits = qe @ km.T
    ex = np.exp(logits - logits.max(-1, keepdims=True))
    wsoft = ex / ex.sum(-1, keepdims=True)
    hq = qe @ inputs["pred_w1"][V:, :].astype(f32) + inputs["pred_b1"].astype(f32)
    esig = 1.0 / (1.0 + np.exp(-(qae @ inputs["erase_w"].astype(f32) + inputs["erase_b"].astype(f32))))
    atanh = np.tanh(qae @ inputs["add_w"].astype(f32) + inputs["add_b"].astype(f32))

    ea = np.zeros((NQA, EAW), np.float16)
    ea[:, 0:V] = esig.astype(np.float16)
    ea[:, 256:256 + V] = atanh.astype(np.float16)
    wh = np.zeros((NQ, WHW), np.float16)
    wh[:, 0:M] = wsoft.astype(np.float16)
    wh[:, 64:64 + FC] = hq.astype(np.float16)

    w1r = inputs["pred_w1"][:V, :].astype(np.float16).reshape(2, 100, FC)
    w2rep = np.tile(inputs["pred_w2"][:, 0].astype(np.float16)[None, :], (P, 1))
    b2rep = np.full((P, 1), inputs["pred_b2"][0], np.float32)
    mv_init = inputs["init_value_memory"].astype(np.float16).reshape(1, -1)
    return dict(ea_table=ea, wh_table=wh, w1r=w1r, w2rep=w2rep, b2rep=b2rep,
                mv_init=mv_init)


def kernel(**inputs):
    inputs = {k: np.asarray(v) for k, v in inputs.items()}
    steps = int(os.environ.get("KERNEL_STEPS", S))

    if steps not in _prog_cache:
        _prog_cache[steps] = _build_program(steps)
    nc = _prog_cache[steps]

    shared = _host_tables(inputs)
    q = inputs["q_data"].astype(np.int64)
    qa = inputs["qa_data"].astype(np.int64)

    in_maps = []
    for core in range(NCORES):
        qs = q[core * BL:(core + 1) * BL]       # [256, S]
        qas = qa[core * BL:(core + 1) * BL]
        # gather order: block g, step k, tile tl, partition p
        def order(x):
            xt = x.T.reshape(S, NT, P)
            return xt.reshape(NBLK, KSTEPS, NT, P).reshape(-1)
        m = dict(shared)
        m["qidx"] = _wrap_idx(order(qs))
        m["qaidx"] = _wrap_idx(order(qas))
        in_maps.append(m)

    trace = bool(int(os.environ.get("KERNEL_TRACE", "0")))
    res = run_bass_kernel_spmd(nc, in_maps, core_ids=list(range(NCORES)), trace=trace)
    global LAST_RESULTS
    LAST_RESULTS = res
    preds = np.concatenate(
        [res.results[i]["preds_out"] for i in range(NCORES)], axis=0
    ).astype(np.float32)
    z = np.zeros_like(preds)
    return (preds, z, z, z)
